# revision 2
# baseline (speedup 1.0000x reference)
"""Trainium2 Bass kernel for nn_AttnLayer_80178449482249 (sparse chunked attention).

Strategy: shard the token axis across 8 NeuronCores (1024 own tokens + a
64-token halo of the previous shard, materialized on the host so no
device-side collectives are needed). Weights are replicated. All matmuls run
as float32r (full-rate fp32 on the PE at N>=256) with fp32 PSUM accumulation.

Layouts (chosen so every matmul operand is in its natural [partition, free]
layout with zero on-device transposes outside attention):
  - activations feature-major ("d-major"): [feature, token]
  - v and the post-softmax attention weights token-major
  - all weights pre-transposed/tiled on the host
RoPE is applied in the "NeoX" half-split form after folding a deinterleave
permutation of the 512-dim q/k space into Wq/Wk rows (and Wk columns); the
1/sqrt(d) score scale is folded into q's RoPE tables.
"""

import os
import sys
import types
import contextlib

import numpy as np

# ---------------------------------------------------------------- dims
T, XD, RED, CS = 8192, 4096, 8, 64
DK = XD // RED            # 512
NCORE = 8
TC = T // NCORE           # 1024 own tokens per core
TH = TC + CS              # 1088 incl. halo
NCH = TC // CS            # 16 chunks per core
KT = XD // 128            # 32 k-tiles over the 4096 dim
DT = DK // 128            # 4 k-tiles over the 512 dim
NEG = -1.0e30

_NC_CACHE = {}
LAST_EXEC_NS = None
LAST_TRACE = None


# ------------------------------------------------------- profiling hook
def _install_ntff_hook():
    """Best-effort injection of the missing antenv.axon_hooks module so
    run_bass_kernel_spmd(trace=True) can capture NTFF profiles."""
    try:
        import antenv.axon_hooks  # noqa: F401
        return
    except ImportError:
        pass
    try:
        import antenv  # noqa: F401
        mod = types.ModuleType("antenv.axon_hooks")
        _state = {"hook": None}

        def set_axon_ntff_profile_hook(h):
            _state["hook"] = h

        def get_axon_ntff_profile_hook():
            return _state["hook"]

        mod.set_axon_ntff_profile_hook = set_axon_ntff_profile_hook
        mod.get_axon_ntff_profile_hook = get_axon_ntff_profile_hook
        sys.modules["antenv.axon_hooks"] = mod

        site = os.environ.get("AXON_SITE_DIR", "/root/.axon_site")
        if site not in sys.path and os.path.isdir(site):
            sys.path.insert(0, site)
        from trn_agent_boot.trn_boot import _ntff_profile_via_ctypes

        so = os.path.join(site, "axon", "libaxon_pjrt.so")
        if not os.path.isfile(so):
            so = "/opt/axon/libaxon_pjrt.so"
        if os.path.isfile(so):
            hook = _ntff_profile_via_ctypes(so)
            if hook is not None:
                set_axon_ntff_profile_hook(hook)
    except Exception:
        pass


# ------------------------------------------------------- device kernel
def _build_nc():
    import concourse.bacc as bacc
    import concourse.mybir as mybir
    import concourse.tile as tile

    dt = mybir.dt
    F = dt.float32
    FR = dt.float32r
    AF = mybir.ActivationFunctionType
    AX = mybir.AxisListType

    nc = bacc.Bacc("TRN2", target_bir_lowering=False, debug=False,
                   num_devices=NCORE)

    xs_t = nc.dram_tensor("xs_t", [KT, 128, TH], FR, kind="ExternalInput").ap()
    wq = nc.dram_tensor("wq", [KT, 128, DK], FR, kind="ExternalInput").ap()
    wk = nc.dram_tensor("wk", [DT, 128, DK], FR, kind="ExternalInput").ap()
    wv = nc.dram_tensor("wv", [KT, 128, XD], FR, kind="ExternalInput").ap()
    wo = nc.dram_tensor("wo", [KT, 128, XD], FR, kind="ExternalInput").ap()
    wr = nc.dram_tensor("wr", [KT, 128, XD], FR, kind="ExternalInput").ap()
    ropes = nc.dram_tensor("ropes", [12, 128, TH], F, kind="ExternalInput").ap()
    mask = nc.dram_tensor("mask", [CS, 2 * CS], F, kind="ExternalInput").ap()
    ident = nc.dram_tensor("ident", [128, 128], F, kind="ExternalInput").ap()
    outd = nc.dram_tensor("outd", [KT, 128, TC], F, kind="ExternalOutput").ap()

    qr_d = nc.dram_tensor("qr_d", [DT, 128, TH], FR).ap()
    krlo_d = nc.dram_tensor("krlo_d", [DT, 128, TH], FR).ap()
    krhi_d = nc.dram_tensor("krhi_d", [DT, 128, TH], FR).ap()
    vs_d = nc.dram_tensor("vs_d", [TH, XD], FR).ap()
    ot_d = nc.dram_tensor("ot_d", [KT, 128, TC], F).ap()

    with tile.TileContext(nc) as tc:
        with tc.tile_pool(name="glob", bufs=1) as glob:
            mask_sb = glob.tile([CS, 2 * CS], F, tag="mask")
            nc.sync.dma_start(mask_sb[:], mask[:])
            ident_sb = glob.tile([128, 128], F, tag="ident")
            nc.sync.dma_start(ident_sb[:], ident[:])

            # ---------------- phase A: q/k projections + RoPE -> DRAM
            with tc.tile_pool(name="phA", bufs=1) as pa, \
                 tc.tile_pool(name="psA", bufs=4, space="PSUM") as psA:
                wq_sb = []
                for k in range(KT):
                    wqt = pa.tile([128, DK], FR, tag=f"wq{k}")
                    nc.sync.dma_start(wqt[:], wq[k])
                    wq_sb.append(wqt)
                wk_sb = []
                for k in range(DT):
                    wkt = pa.tile([128, DK], FR, tag=f"wk{k}")
                    nc.sync.dma_start(wkt[:], wk[k])
                    wk_sb.append(wkt)

                blocks = [(0, 256), (256, 256), (512, 256), (768, 256),
                          (1024, 64)]
                for s, w in blocks:
                    xb = []
                    for k in range(KT):
                        xt = pa.tile([128, w], FR, tag="xb", bufs=40,
                                     padded_shape=[128, 256])
                        nc.sync.dma_start(xt[:], xs_t[k, :, s:s + w])
                        xb.append(xt)
                    qs_sb = []
                    for m in range(DT):
                        ps = psA.tile([128, w], F, tag="mm",
                                      padded_shape=[128, 512])
                        for k in range(KT):
                            nc.tensor.matmul(
                                ps[:], wq_sb[k][:, m * 128:(m + 1) * 128],
                                xb[k],
                                start=(k == 0), stop=(k == KT - 1))
                        qt = pa.tile([128, w], FR, tag=f"qs{m}", bufs=2,
                                     padded_shape=[128, 256])
                        nc.vector.tensor_copy(qt[:], ps[:])
                        qs_sb.append(qt)
                    ks_sb = []
                    for e in range(DT):
                        ps = psA.tile([128, w], F, tag="mm",
                                      padded_shape=[128, 512])
                        for d2 in range(DT):
                            nc.tensor.matmul(
                                ps[:], wk_sb[d2][:, e * 128:(e + 1) * 128],
                                qs_sb[d2],
                                start=(d2 == 0), stop=(d2 == DT - 1))
                        kt_ = pa.tile([128, w], F, tag=f"ks{e}", bufs=2,
                                      padded_shape=[128, 256])
                        nc.vector.tensor_copy(kt_[:], ps[:])
                        ks_sb.append(kt_)
                    # rope tables for this block
                    r_sb = []
                    for i in range(12):
                        rt = pa.tile([128, w], F, tag=f"rope{i}", bufs=2,
                                     padded_shape=[128, 256])
                        nc.sync.dma_start(rt[:], ropes[i, :, s:s + w])
                        r_sb.append(rt)

                    def rope_out(src, ci, si, dest_dram):
                        # src: list of 4 d-tiles; tables: cos r_sb[ci..ci+1],
                        # sin r_sb[si..si+1]
                        for m in range(4):
                            half = m % 2
                            cos_t = r_sb[ci + half]
                            sin_t = r_sb[si + half]
                            t1 = pa.tile([128, w], F, tag="tmp1", bufs=4,
                                         padded_shape=[128, 256])
                            t2 = pa.tile([128, w], F, tag="tmp2", bufs=4,
                                         padded_shape=[128, 256])
                            nc.vector.tensor_mul(t1[:], src[m][:], cos_t[:])
                            nc.vector.tensor_mul(t2[:], src[(m + 2) % 4][:],
                                                 sin_t[:])
                            ot = pa.tile([128, w], FR, tag="ropeout", bufs=6,
                                         padded_shape=[128, 256])
                            if m < 2:
                                nc.vector.tensor_sub(ot[:], t1[:], t2[:])
                            else:
                                nc.vector.tensor_add(ot[:], t1[:], t2[:])
                            nc.sync.dma_start(dest_dram[m, :, s:s + w], ot[:])

                    rope_out(qs_sb, 0, 2, qr_d)
                    rope_out(ks_sb, 4, 6, krlo_d)
                    rope_out(ks_sb, 8, 10, krhi_d)

            # ---------------- phase C: v projection (token-major) -> DRAM
            with tc.tile_pool(name="phC", bufs=1) as pc, \
                 tc.tile_pool(name="psC", bufs=4, space="PSUM") as psC:
                halves = [(0, 576), (576, 512)]
                for hs, hw in halves:
                    xh = []
                    for k in range(KT):
                        xt = pc.tile([128, hw], FR, tag=f"xh{k}",
                                     padded_shape=[128, 576])
                        nc.sync.dma_start(xt[:], xs_t[k, :, hs:hs + hw])
                        xh.append(xt)
                    ntt = (hw + 127) // 128
                    for vb in range(XD // 512):
                        wv_sb = []
                        for k in range(KT):
                            wt = pc.tile([128, 512], FR, tag="wv", bufs=36)
                            nc.sync.dma_start(
                                wt[:], wv[k, :, vb * 512:(vb + 1) * 512])
                            wv_sb.append(wt)
                        for tt in range(ntt):
                            tw = min(128, hw - tt * 128)
                            ps = psC.tile([tw, 512], F, tag="mm",
                                          padded_shape=[128, 512])
                            for k in range(KT):
                                nc.tensor.matmul(
                                    ps[:],
                                    xh[k][:, tt * 128:tt * 128 + tw],
                                    wv_sb[k],
                                    start=(k == 0), stop=(k == KT - 1))
                            vo = pc.tile([tw, 512], FR, tag="vout", bufs=4,
                                         padded_shape=[128, 512])
                            nc.vector.tensor_copy(vo[:], ps[:])
                            nc.sync.dma_start(
                                vs_d[hs + tt * 128:hs + tt * 128 + tw,
                                     vb * 512:(vb + 1) * 512], vo[:])

            # ---------------- ys pool lives through phases B and D
            with tc.tile_pool(name="ys", bufs=1) as ysp:
                ys_sb = []
                for u in range(KT):
                    yt = ysp.tile([128, TC], FR, tag=f"ys{u}")
                    ys_sb.append(yt)

                # ------------ phase B: chunked attention, ys stays in SBUF
                with tc.tile_pool(name="phB", bufs=1) as pb, \
                     tc.tile_pool(name="psS", bufs=2, space="PSUM") as psS, \
                     tc.tile_pool(name="psT", bufs=2, space="PSUM") as psT, \
                     tc.tile_pool(name="psY", bufs=4, space="PSUM") as psY:
                    for j in range(NCH):
                        qt = []
                        for m in range(DT):
                            q1 = pb.tile([128, CS], FR, tag=f"aq{m}", bufs=2)
                            nc.sync.dma_start(
                                q1[:], qr_d[m, :, CS + CS * j:2 * CS + CS * j])
                            qt.append(q1)
                        kt_ = []
                        for m in range(DT):
                            k1 = pb.tile([128, 2 * CS], FR, tag=f"ak{m}", bufs=2)
                            nc.sync.dma_start(
                                k1[:, 0:CS], krlo_d[m, :, CS * j:CS * j + CS])
                            nc.sync.dma_start(
                                k1[:, CS:2 * CS],
                                krhi_d[m, :, CS * j + CS:CS * j + 2 * CS])
                            kt_.append(k1)
                        vt = pb.tile([128, XD], FR, tag="av", bufs=2)
                        nc.sync.dma_start(vt[:], vs_d[CS * j:CS * j + 2 * CS, :])

                        ps_s = psS.tile([CS, 2 * CS], F, tag="s")
                        for m in range(DT):
                            nc.tensor.matmul(ps_s[:], qt[m],
                                             kt_[m],
                                             start=(m == 0), stop=(m == DT - 1))
                        s_sb = pb.tile([CS, 2 * CS], F, tag="s_sb", bufs=2)
                        nc.vector.tensor_add(s_sb[:], ps_s[:], mask_sb[:])
                        nmax = pb.tile([CS, 1], F, tag="nmax", bufs=2)
                        nc.vector.reduce_max(nmax[:], s_sb[:], AX.X, negate=True)
                        e_sb = pb.tile([CS, 2 * CS], F, tag="e_sb", bufs=2)
                        rsum = pb.tile([CS, 1], F, tag="rsum", bufs=2)
                        nc.scalar.activation(e_sb[:], s_sb[:], AF.Exp,
                                             bias=nmax[:], accum_out=rsum[:])
                        rinv = pb.tile([CS, 1], F, tag="rinv", bufs=2)
                        nc.vector.reciprocal(rinv[:], rsum[:])
                        a_sb = pb.tile([CS, 2 * CS], F, tag="a_sb", bufs=2)
                        nc.vector.tensor_scalar_mul(a_sb[:], e_sb[:], rinv[:])
                        ps_t = psT.tile([2 * CS, CS], F, tag="at")
                        nc.tensor.transpose(ps_t[:], a_sb[:],
                                            ident_sb[0:CS, 0:CS])
                        at_sb = pb.tile([2 * CS, CS], FR, tag="at_sb", bufs=2)
                        nc.vector.tensor_copy(at_sb[:], ps_t[:])
                        for u in range(KT):
                            ps_y = psY.tile([128, CS], F, tag="yp")
                            nc.tensor.matmul(
                                ps_y[:], vt[:, u * 128:(u + 1) * 128],
                                at_sb, start=True, stop=True)
                            nc.vector.tensor_copy(
                                ys_sb[u][:, CS * j:CS * (j + 1)], ps_y[:])

                # ------------ phase D: out_t = Wo @ ys -> DRAM staging
                with tc.tile_pool(name="phD", bufs=1) as pd_, \
                     tc.tile_pool(name="psD", bufs=4, space="PSUM") as psD:
                    for og in range(XD // 256):
                        wo_sb = []
                        for k in range(KT):
                            wt = pd_.tile([128, 256], FR, tag="wo", bufs=40)
                            nc.sync.dma_start(
                                wt[:], wo[k, :, og * 256:(og + 1) * 256])
                            wo_sb.append(wt)
                        for oi in range(2):
                            ot_i = og * 2 + oi
                            for tb in range(TC // 512):
                                ps = psD.tile([128, 512], F, tag="mm")
                                for u in range(KT):
                                    nc.tensor.matmul(
                                        ps[:],
                                        wo_sb[u][:, oi * 128:(oi + 1) * 128],
                                        ys_sb[u][:, tb * 512:(tb + 1) * 512],
                                        start=(u == 0), stop=(u == KT - 1))
                                oo = pd_.tile([128, 512], F, tag="dout", bufs=4)
                                nc.vector.tensor_copy(oo[:], ps[:])
                                nc.sync.dma_start(
                                    ot_d[ot_i, :, tb * 512:(tb + 1) * 512],
                                    oo[:])

            # ---------------- phase E: gate with sigmoid(Wr @ xs), emit out
            with tc.tile_pool(name="phE", bufs=1) as pe, \
                 tc.tile_pool(name="psE", bufs=4, space="PSUM") as psE:
                xso = []
                for k in range(KT):
                    xt = pe.tile([128, TC], FR, tag=f"xso{k}")
                    nc.sync.dma_start(xt[:], xs_t[k, :, CS:TH])
                    xso.append(xt)
                for og in range(XD // 256):
                    wr_sb = []
                    for k in range(KT):
                        wt = pe.tile([128, 256], FR, tag="wr", bufs=40)
                        nc.sync.dma_start(
                            wt[:], wr[k, :, og * 256:(og + 1) * 256])
                        wr_sb.append(wt)
                    for oi in range(2):
                        ot_i = og * 2 + oi
                        for tb in range(TC // 512):
                            ps = psE.tile([128, 512], F, tag="mm")
                            for u in range(KT):
                                nc.tensor.matmul(
                                    ps[:],
                                    wr_sb[u][:, oi * 128:(oi + 1) * 128],
                                    xso[u][:, tb * 512:(tb + 1) * 512],
                                    start=(u == 0), stop=(u == KT - 1))
                            sg = pe.tile([128, 512], F, tag="sg", bufs=4)
                            nc.scalar.activation(sg[:], ps[:], AF.Sigmoid)
                            od = pe.tile([128, 512], F, tag="odin", bufs=4)
                            nc.sync.dma_start(
                                od[:], ot_d[ot_i, :, tb * 512:(tb + 1) * 512])
                            fin = pe.tile([128, 512], F, tag="fin", bufs=4)
                            nc.vector.tensor_mul(fin[:], od[:], sg[:])
                            nc.sync.dma_start(
                                outd[ot_i, :, tb * 512:(tb + 1) * 512], fin[:])

    nc.compile()
    return nc


def _get_nc():
    if "nc" not in _NC_CACHE:
        _NC_CACHE["nc"] = _build_nc()
    return _NC_CACHE["nc"]


# ------------------------------------------------------- host-side prep
def _host_prep(xs, Wq, Wk, Wv, Wo, Wr):
    f = np.float32
    xs = np.asarray(xs, f)
    Wq = np.asarray(Wq, f)
    Wk = np.asarray(Wk, f)
    Wv = np.asarray(Wv, f)
    Wo = np.asarray(Wo, f)
    Wr = np.asarray(Wr, f)

    perm = np.concatenate([np.arange(0, DK, 2), np.arange(1, DK, 2)])
    WqP = Wq[perm, :]
    WkP = Wk[np.ix_(perm, perm)]

    wq_h = np.ascontiguousarray(WqP.T).reshape(KT, 128, DK)
    wk_h = np.ascontiguousarray(WkP.T).reshape(DT, 128, DK)
    wv_h = np.ascontiguousarray(Wv.T).reshape(KT, 128, XD)
    wo_h = np.ascontiguousarray(Wo.T).reshape(KT, 128, XD)
    wr_h = np.ascontiguousarray(Wr.T).reshape(KT, 128, XD)

    inv = 10000.0 ** (-np.arange(0, DK, 2, dtype=np.float64) / DK)
    ang = np.arange(2 * CS, dtype=np.float64)[:, None] * inv[None, :]
    cosv = np.cos(ang)
    sinv = np.sin(ang)
    scale = 1.0 / np.sqrt(np.float64(DK))

    def dmaj(tab):  # [npos, 256] -> [2, 128, npos]
        return np.ascontiguousarray(tab.T.astype(f)).reshape(2, 128, -1)

    reps = TH // CS
    tabs = [dmaj(cosv[CS:] * scale), dmaj(sinv[CS:] * scale),
            dmaj(cosv[:CS]), dmaj(sinv[:CS]),
            dmaj(cosv[CS:]), dmaj(sinv[CS:])]
    ropes = np.concatenate([np.tile(t, (1, 1, reps)) for t in tabs], axis=0)
    ropes = np.ascontiguousarray(ropes, f)

    ii = np.arange(CS)[:, None]
    jj = np.arange(2 * CS)[None, :]
    mask = np.where(jj <= ii + CS, 0.0, NEG).astype(f)
    ident = np.eye(128, dtype=f)

    xsT = np.ascontiguousarray(xs.T)  # [XD, T]
    shards = []
    for c in range(NCORE):
        lo = c * TC - CS
        if lo < 0:
            blk = np.zeros((XD, TH), f)
            blk[:, CS:] = xsT[:, :TC]
        else:
            blk = xsT[:, lo:lo + TH]
        shards.append(np.ascontiguousarray(blk).reshape(KT, 128, TH))

    common = {"wq": wq_h, "wk": wk_h, "wv": wv_h, "wo": wo_h, "wr": wr_h,
              "ropes": ropes, "mask": mask, "ident": ident}
    in_maps = [dict(common, xs_t=shards[c]) for c in range(NCORE)]
    return in_maps


# ------------------------------------------------------- entry point
def kernel(xs, Wq, Wk, Wv, Wo, Wr, trace=False):
    global LAST_EXEC_NS, LAST_TRACE
    if trace:
        _install_ntff_hook()
    from concourse.bass_utils import run_bass_kernel_spmd

    nc = _get_nc()
    in_maps = _host_prep(xs, Wq, Wk, Wv, Wo, Wr)
    res = run_bass_kernel_spmd(nc, in_maps, core_ids=list(range(NCORE)),
                               trace=trace)
    LAST_EXEC_NS = res.exec_time_ns
    LAST_TRACE = (res.instructions_and_trace[1]
                  if res.instructions_and_trace else None)

    out = np.empty((T, XD), np.float32)
    for c in range(NCORE):
        blk = res.results[c]["outd"].reshape(XD, TC)  # d-major [4096, 1024]
        out[c * TC:(c + 1) * TC, :] = blk.T
    return out
